# revision 1
# baseline (speedup 1.0000x reference)
"""Trainium2 Bass kernel for nn_CrystalNet (dense transformer, 8 NeuronCores).

Sharding: cyclic over sequence for the 28-iteration recurrence (core c owns
tokens t == c (mod 8) of both batches; per-iteration AllGather of k/v), with
the lm_head vocab-sharded 8 ways and reassembled on host.
All matmuls bf16 with fp32 PSUM accumulation.
"""
import math
import sys

import numpy as np

sys.path.insert(0, "/opt/trn_rl_repo")

B, S, D, H, IT, V = 2, 1024, 1024, 8, 28, 50257
P = 128
DH = D // H          # 128
NDT = D // P         # 8 feature tiles
NC = 8               # cores
Q = S // NC          # 128 tokens per core per batch
TPC = B * Q          # 256 tokens per core
VS = 6284            # padded vocab shard (8*6284 = 50272 >= V)
NVB = (VS + 511) // 512   # 13 vocab blocks per shard
EPS = 1.1920929e-07
ATT_SCALE = -1.0 / math.sqrt(DH)

_CACHE = {}


def _build(step_size, its=IT, with_lm=True, no_collectives=False):
    import concourse.bacc as bacc
    import concourse.tile as tile
    import concourse.mybir as mybir

    f32 = mybir.dt.float32
    bf16 = mybir.dt.bfloat16
    AF = mybir.ActivationFunctionType
    ALU = mybir.AluOpType

    nc = bacc.Bacc("TRN2", target_bir_lowering=False, debug=False, num_devices=NC)

    # ---- DRAM I/O (per core) ----
    x0T_in = nc.dram_tensor("x0T", [P, NDT, TPC], f32, kind="ExternalInput")
    wk_in = nc.dram_tensor("wkT", [NDT, P, D], bf16, kind="ExternalInput")
    wv_in = nc.dram_tensor("wvT", [NDT, P, D], bf16, kind="ExternalInput")
    ws_in = nc.dram_tensor("wsT", [NDT, P, D], bf16, kind="ExternalInput")
    wo_in = nc.dram_tensor("woT", [NDT, P, D], bf16, kind="ExternalInput")
    nwg_in = nc.dram_tensor("nwgc", [P, NDT, IT], f32, kind="ExternalInput")
    beta_in = nc.dram_tensor("beta", [P, NDT, IT], f32, kind="ExternalInput")
    onw_in = nc.dram_tensor("onwc", [P, NDT, 1], f32, kind="ExternalInput")
    mask_in = nc.dram_tensor("mask3", [P, NC, Q], bf16, kind="ExternalInput")
    onec_in = nc.dram_tensor("onec", [P, 1], bf16, kind="ExternalInput")
    if with_lm:
        lm_in = nc.dram_tensor("lmT", [NDT, P, VS], bf16, kind="ExternalInput")
        out_ext = nc.dram_tensor("out", [2 * NC * P, VS], bf16, kind="ExternalOutput")
    else:
        out_ext = nc.dram_tensor("out", [P, NDT, TPC], f32, kind="ExternalOutput")

    with tile.TileContext(nc) as tc:
        with (
            tc.tile_pool(name="res", bufs=1) as res,      # persistent residents
            tc.tile_pool(name="ps", bufs=2, space="PSUM") as ps,
            tc.tile_pool(name="dram", bufs=2, space="DRAM") as dram,
        ):
            # ---- load residents ----
            wk_sb = res.tile([P, NDT, D], bf16)
            wv_sb = res.tile([P, NDT, D], bf16)
            ws_sb = res.tile([P, NDT, D], bf16)
            wo_sb = res.tile([P, NDT, D], bf16)
            for w_sb, w_in in ((wk_sb, wk_in), (wv_sb, wv_in), (ws_sb, ws_in), (wo_sb, wo_in)):
                nc.sync.dma_start(out=w_sb[:], in_=w_in.ap().rearrange("a p f -> p a f"))
            x_sb = res.tile([P, NDT, TPC], f32)
            nc.sync.dma_start(out=x_sb[:], in_=x0T_in[:])
            beta_sb = res.tile([P, NDT, IT], f32)
            nc.sync.dma_start(out=beta_sb[:], in_=beta_in[:])
            nwgc_sb = res.tile([P, NDT, IT], f32)
            nc.sync.dma_start(out=nwgc_sb[:], in_=nwg_in[:])
            onwc_sb = res.tile([P, NDT, 1], f32)
            nc.sync.dma_start(out=onwc_sb[:], in_=onw_in[:])
            mask_sb = res.tile([P, NC, Q], bf16)
            nc.sync.dma_start(out=mask_sb[:], in_=mask_in[:])
            onec_sb = res.tile([P, 1], bf16)
            nc.sync.dma_start(out=onec_sb[:], in_=onec_in[:])
            oner_f = res.tile([1, P], f32)
            nc.vector.memset(oner_f[:], 1.0)
            epsc = res.tile([1, 1], f32)
            nc.vector.memset(epsc[:], EPS)

            rg = [list(range(NC))]
            if with_lm:
                agy_out = dram.tile([NC, P, NDT, TPC], bf16, tag="agyo", bufs=1,
                                    addr_space="Shared", name="agy_out")
            else:
                agy_out = None

            with tc.tile_pool(name="stg", bufs=2) as stg:
                def norm_to(col_sb, it_or_none, dst_sb):
                    """dst = x * rsqrt(mean(x^2)+eps) * col [+ beta[:, :, it]]"""
                    sq = stg.tile([P, NDT, TPC], bf16, tag="sqz")
                    nc.vector.tensor_mul(sq[:], x_sb[:], x_sb[:])
                    ssq = ps.tile([1, TPC], f32, tag="small", bufs=2)
                    for dt in range(NDT):
                        nc.tensor.matmul(ssq[:], onec_sb[:], sq[:, dt, :],
                                         start=(dt == 0), stop=(dt == NDT - 1))
                    lnv = stg.tile([1, TPC], f32, tag="lnv")
                    nc.scalar.activation(lnv[:], ssq[:], AF.Ln, scale=1.0 / D, bias=epsc[:])
                    rstd = stg.tile([1, TPC], f32, tag="rstd")
                    nc.scalar.activation(rstd[:], lnv[:], AF.Exp, scale=-0.5)
                    sb = ps.tile([P, TPC], f32, tag="small", bufs=2)
                    nc.tensor.matmul(sb[:], oner_f[:], rstd[:], start=True, stop=True)
                    col2 = col_sb if it_or_none is not None else None
                    for dt in range(NDT):
                        sc_ap = (col_sb[:, dt, it_or_none:it_or_none + 1]
                                 if it_or_none is not None else col_sb[:, dt, 0:1])
                        nc.vector.scalar_tensor_tensor(dst_sb[:, dt, :], x_sb[:, dt, :],
                                                       sc_ap, sb[:], ALU.mult, ALU.mult)
                        if it_or_none is not None:
                            nc.vector.tensor_scalar_add(
                                dst_sb[:, dt, :], dst_sb[:, dt, :],
                                beta_sb[:, dt, it_or_none:it_or_none + 1])

                for it in range(its):
                    hT = stg.tile([P, NDT, TPC], bf16, tag="hT", bufs=1)
                    norm_to(nwgc_sb, it % IT, hT)

                    # ---- projections ----
                    kst = stg.tile([P, NDT, TPC], bf16, tag="kst")
                    for ft in range(NDT):
                        kp = ps.tile([P, TPC], f32, tag="proj", bufs=3)
                        for dt in range(NDT):
                            nc.tensor.matmul(kp[:], wk_sb[:, dt, ft * P:(ft + 1) * P],
                                             hT[:, dt, :], start=(dt == 0), stop=(dt == NDT - 1))
                        (nc.vector.tensor_copy if ft % 2 else nc.scalar.copy)(kst[:, ft, :], kp[:])
                    agk_in = dram.tile([P, NDT, TPC], bf16, tag="agki")
                    agk_out = dram.tile([NC, P, NDT, TPC], bf16, tag="agko", addr_space="Shared")
                    nc.sync.dma_start(out=agk_in[:], in_=kst[:])
                    if no_collectives:
                        nc.sync.dma_start(out=agk_out[0], in_=agk_in[:])
                    else:
                        nc.gpsimd.collective_compute("AllGather", ALU.bypass, replica_groups=rg,
                                                     ins=[agk_in.opt()], outs=[agk_out.opt()])

                    vst = stg.tile([P, B, D], bf16, tag="vst")
                    for tt in range(B):
                        for fb in range(2):
                            vp = ps.tile([P, 512], f32, tag="proj", bufs=3)
                            for dt in range(NDT):
                                nc.tensor.matmul(vp[:], hT[:, dt, tt * P:(tt + 1) * P],
                                                 wv_sb[:, dt, fb * 512:(fb + 1) * 512],
                                                 start=(dt == 0), stop=(dt == NDT - 1))
                            (nc.vector.tensor_copy if fb else nc.scalar.copy)(
                                vst[:, tt, fb * 512:(fb + 1) * 512], vp[:])
                    agv_in = dram.tile([P, B, D], bf16, tag="agvi")
                    agv_out = dram.tile([NC, P, B, D], bf16, tag="agvo", addr_space="Shared")
                    nc.sync.dma_start(out=agv_in[:], in_=vst[:])
                    if no_collectives:
                        nc.sync.dma_start(out=agv_out[0], in_=agv_in[:])
                    else:
                        nc.gpsimd.collective_compute("AllGather", ALU.bypass, replica_groups=rg,
                                                     ins=[agv_in.opt()], outs=[agv_out.opt()])

                    sT = stg.tile([P, NDT, TPC], bf16, tag="sT")
                    for ft in range(NDT):
                        sp = ps.tile([P, TPC], f32, tag="proj", bufs=3)
                        for dt in range(NDT):
                            nc.tensor.matmul(sp[:], ws_sb[:, dt, ft * P:(ft + 1) * P],
                                             hT[:, dt, :], start=(dt == 0), stop=(dt == NDT - 1))
                        (nc.vector.tensor_copy if ft % 2 else nc.scalar.copy)(sT[:, ft, :], sp[:])

                    # ---- attention (batch-serial; kg/vg single-buffered) ----
                    zT = stg.tile([P, NDT, TPC], bf16, tag="sqz")
                    for b in range(B):
                        kg = stg.tile([P, H, NC, Q], bf16, tag="kg", bufs=1)
                        vg = stg.tile([P, NC, H, DH], bf16, tag="vg", bufs=1)
                        for r in range(NC):
                            nc.sync.dma_start(out=kg[:, :, r, :],
                                              in_=agk_out[r, :, :, b * Q:(b + 1) * Q])
                            nc.sync.dma_start(
                                out=vg[:, r, :, :],
                                in_=agv_out[r, :, b, :].rearrange("p (h d) -> p h d", h=H))
                        for hg in range(2):
                            pv = ps.tile([P, 4, Q], f32, tag="att", bufs=3)
                            lp = ps.tile([1, 4, Q], f32, tag="small", bufs=2)
                            for h4 in range(4):
                                h = hg * 4 + h4
                                et = stg.tile([P, NC, Q], bf16, tag="et", bufs=2)
                                for sg in range(2):
                                    ep = ps.tile([P, 4, Q], f32, tag="att", bufs=3)
                                    for r4 in range(4):
                                        r = sg * 4 + r4
                                        nc.tensor.matmul(ep[:, r4, :], kg[:, h, r, :],
                                                         kst[:, h, b * Q:(b + 1) * Q],
                                                         start=True, stop=True)
                                    nc.scalar.activation(et[:, sg * 4:(sg + 1) * 4, :], ep[:],
                                                         AF.Exp, scale=ATT_SCALE)
                                nc.vector.tensor_mul(et[:], et[:], mask_sb[:])
                                for r in range(NC):
                                    nc.tensor.matmul(lp[0:1, h4, :], onec_sb[:], et[:, r, :],
                                                     start=(r == 0), stop=(r == NC - 1))
                                for r in range(NC):
                                    nc.tensor.matmul(pv[:, h4, :], vg[:, r, h, :], et[:, r, :],
                                                     start=(r == 0), stop=(r == NC - 1))
                            rl = stg.tile([1, 4, Q], f32, tag="rl")
                            nc.vector.reciprocal(rl[:], lp[:])
                            sps = ps.tile([P, 4, Q], f32, tag="small", bufs=2)
                            nc.tensor.matmul(sps[:], oner_f[:], rl[0:1, :, :],
                                             start=True, stop=True)
                            ssb = stg.tile([P, 4, Q], f32, tag="ssb")
                            nc.vector.tensor_copy(ssb[:], sps[:])
                            for h4 in range(4):
                                nc.vector.tensor_mul(zT[:, hg * 4 + h4, b * Q:(b + 1) * Q],
                                                     pv[:, h4, :], ssb[:, h4, :])
                    nc.vector.tensor_add(zT[:], zT[:], sT[:])

                    # ---- Wo + residual update ----
                    st_f = float(step_size[it % IT])
                    for ft in range(NDT):
                        fp = ps.tile([P, TPC], f32, tag="proj", bufs=3)
                        for mt in range(NDT):
                            nc.tensor.matmul(fp[:], wo_sb[:, mt, ft * P:(ft + 1) * P],
                                             zT[:, mt, :], start=(mt == 0), stop=(mt == NDT - 1))
                        nc.vector.scalar_tensor_tensor(x_sb[:, ft, :], fp[:], st_f,
                                                       x_sb[:, ft, :], ALU.mult, ALU.add)

                if with_lm:
                    yst = stg.tile([P, NDT, TPC], bf16, tag="hT", bufs=1)
                    norm_to(onwc_sb, None, yst)
                    agy_in = dram.tile([P, NDT, TPC], bf16, tag="agyi", bufs=1)
                    nc.sync.dma_start(out=agy_in[:], in_=yst[:])
                    nc.gpsimd.collective_compute("AllGather", ALU.bypass, replica_groups=rg,
                                                 ins=[agy_in.opt()], outs=[agy_out.opt()])
                else:
                    nc.sync.dma_start(out=out_ext[:], in_=x_sb[:])

            if with_lm:
                # ---- lm head (vocab shard streamed from HBM) ----
                with tc.tile_pool(name="lmp", bufs=2) as lmp:
                    y_sb = lmp.tile([P, NDT, NC * TPC], bf16, tag="ysb", bufs=1)
                    for r in range(NC):
                        nc.sync.dma_start(out=y_sb[:, :, r * TPC:(r + 1) * TPC],
                                          in_=agy_out[r])
                    for vb in range(NVB):
                        nv = min(512, VS - vb * 512)
                        lw = lmp.tile([P, NDT, 512], bf16, tag="lw", bufs=2)
                        nc.sync.dma_start(
                            out=lw[:, :, :nv],
                            in_=lm_in.ap()[:, :, vb * 512:vb * 512 + nv]
                                .rearrange("a p v -> p a v"))
                        for ts in range(2 * NC):
                            op = ps.tile([P, 512], f32, tag="proj", bufs=3)
                            for dt in range(NDT):
                                nc.tensor.matmul(op[:, :nv],
                                                 y_sb[:, dt, ts * P:(ts + 1) * P],
                                                 lw[:, dt, :nv],
                                                 start=(dt == 0), stop=(dt == NDT - 1))
                            ob = lmp.tile([P, 512], bf16, tag="ob", bufs=4)
                            (nc.vector.tensor_copy if ts % 2 else nc.scalar.copy)(
                                ob[:, :nv], op[:, :nv])
                            nc.sync.dma_start(
                                out=out_ext[ts * P:(ts + 1) * P, vb * 512:vb * 512 + nv],
                                in_=ob[:, :nv])
    nc.compile()
    return nc


def _make_runner(nc, n_cores):
    import jax
    import jax.numpy as jnp
    from jax.sharding import Mesh, PartitionSpec, NamedSharding
    from jax.experimental.shard_map import shard_map
    import concourse.mybir as mybir
    from concourse.bass2jax import _bass_exec_p, install_neuronx_cc_hook, partition_id_tensor

    install_neuronx_cc_hook()
    partition_name = nc.partition_id_tensor.name if nc.partition_id_tensor else None
    in_names, out_names, out_avals = [], [], []
    for alloc in nc.m.functions[0].allocations:
        if not isinstance(alloc, mybir.MemoryLocationSet):
            continue
        name = alloc.memorylocations[0].name
        if alloc.kind == "ExternalInput":
            if name != partition_name:
                in_names.append(name)
        elif alloc.kind == "ExternalOutput":
            out_names.append(name)
            out_avals.append(jax.core.ShapedArray(tuple(alloc.tensor_shape),
                                                  mybir.dt.np(alloc.dtype)))
    n_params = len(in_names)
    all_in = list(in_names) + list(out_names)
    if partition_name is not None:
        all_in.append(partition_name)
    donate = tuple(range(n_params, n_params + len(out_names)))

    def _body(*args):
        operands = list(args)
        if partition_name is not None:
            operands.append(partition_id_tensor())
        return tuple(_bass_exec_p.bind(
            *operands, out_avals=tuple(out_avals), in_names=tuple(all_in),
            out_names=tuple(out_names), lowering_input_output_aliases=(),
            sim_require_finite=True, sim_require_nnan=True, nc=nc))

    devices = jax.devices()[:n_cores]
    mesh = Mesh(np.asarray(devices), ("core",))
    spec = PartitionSpec("core")
    sharding = NamedSharding(mesh, spec)
    n_out = len(out_names)
    sharded = jax.jit(
        shard_map(_body, mesh=mesh, in_specs=(spec,) * (n_params + n_out),
                  out_specs=(spec,) * n_out, check_rep=False),
        donate_argnums=donate, keep_unused=True)

    zero_fns = [
        jax.jit(lambda av=av: jnp.zeros((n_cores * av.shape[0], *av.shape[1:]), av.dtype),
                out_shardings=sharding)
        for av in out_avals
    ]

    def run(in_maps):
        per_core = [[np.asarray(m[name]) for name in in_names] for m in in_maps]
        concat_in = [np.concatenate([per_core[c][i] for c in range(n_cores)], axis=0)
                     for i in range(n_params)]
        zeros = [zf() for zf in zero_fns]
        outs = sharded(*concat_in, *zeros)
        jax.block_until_ready(outs)
        return [
            {name: np.asarray(outs[i]).reshape(n_cores, *out_avals[i].shape)[c]
             for i, name in enumerate(out_names)}
            for c in range(n_cores)
        ]
    run.in_names = in_names
    run._sharded = sharded
    run._zero_fns = zero_fns
    return run


def _bf16(x):
    """fp32 -> bf16 (round to nearest even)."""
    import ml_dtypes
    x = np.ascontiguousarray(x, dtype=np.float32)
    u = x.view(np.uint32)
    r = ((u >> 16) & 1).astype(np.uint32)
    out = ((u + 0x7FFF + r) >> 16).astype(np.uint16)
    return np.asarray(out.view(ml_dtypes.bfloat16))


def _prep_inputs(inputs):
    tokens = np.asarray(inputs["tokens"])
    embed = np.asarray(inputs["embed"], dtype=np.float32)
    step_size = np.asarray(inputs["step_size"], dtype=np.float32)
    norm_w = np.asarray(inputs["norm_w"], dtype=np.float32)
    gamma = np.asarray(inputs["gamma"], dtype=np.float32)
    beta = np.asarray(inputs["beta"], dtype=np.float32)
    out_norm_w = np.asarray(inputs["out_norm_w"], dtype=np.float32)
    lm_head = np.asarray(inputs["lm_head_w"], dtype=np.float32)

    x0 = embed[tokens]                      # [B, S, D] fp32
    wT = {}
    for nm in ("Wk", "Wv", "Ws", "Wo"):
        w = np.asarray(inputs[nm], dtype=np.float32)
        wT[nm] = _bf16(w.T.reshape(NDT, P, D))   # [dt, p, f] = W[f, dt*P+p]

    nwgc = np.ascontiguousarray(np.transpose(
        (norm_w * gamma).reshape(IT, NDT, P), (2, 1, 0)), dtype=np.float32)  # [p, dt, it]
    beta_c = np.ascontiguousarray(np.transpose(beta.reshape(IT, NDT, P), (2, 1, 0)))  # [p, dt, it]
    onwc = np.ascontiguousarray(
        out_norm_w.reshape(NDT, P).T[:, :, None], dtype=np.float32)  # [p, dt, 1]
    onec = _bf16(np.ones((P, 1), np.float32))

    lm_pad = np.zeros((NC * VS, D), np.float32)
    lm_pad[:V] = lm_head
    lmT_shards = [
        _bf16(lm_pad[c * VS:(c + 1) * VS].T.reshape(NDT, P, VS))
        for c in range(NC)
    ]

    in_maps = []
    for c in range(NC):
        # x0T chunk: [p, dt, s*Q+q] = x0[s, 8q + c, dt*P + p]
        xc = x0[:, c::NC, :]                      # [B, Q, D]
        x0T = np.ascontiguousarray(
            np.transpose(xc.reshape(B * Q, NDT, P), (2, 1, 0)), dtype=np.float32)
        # mask3[jq, r, q] = 1 if 8*jq + r <= 8*q + c
        jq = np.arange(Q)[:, None, None]
        r = np.arange(NC)[None, :, None]
        q = np.arange(Q)[None, None, :]
        m = ((NC * jq + r) <= (NC * q + c)).astype(np.float32)
        in_maps.append(dict(
            x0T=x0T,
            wkT=wT["Wk"], wvT=wT["Wv"], wsT=wT["Ws"], woT=wT["Wo"],
            nwgc=nwgc, beta=beta_c, onwc=onwc,
            mask3=_bf16(m), onec=onec,
            lmT=lmT_shards[c],
        ))
    return in_maps, step_size


def kernel(**inputs):
    in_maps, step_size = _prep_inputs(inputs)
    key = ("full", IT, True)
    if key not in _CACHE:
        nc = _build(step_size, its=IT, with_lm=True)
        _CACHE[key] = (nc, _make_runner(nc, NC))
    nc, run = _CACHE[key]
    results = run(in_maps)

    logits = np.empty((B, S, V), np.float32)
    for c in range(NC):
        vlo = c * VS
        take = min(VS, V - vlo) if vlo < V else 0
        if take <= 0:
            continue
        o = np.asarray(results[c]["out"]).astype(np.float32)   # [2*NC*P, VS]
        o = o.reshape(NC, B, Q, VS)                            # rows ts=(r, s) -> token (s, 8q+r)
        for r in range(NC):
            logits[:, r::NC, vlo:vlo + take] = o[r, :, :, :take]
    return logits

